# revision 20
# baseline (speedup 1.0000x reference)
"""Trainium2 Bass kernel for PolarProjectionDepth.

reference computation (per batch b):
  scores[h,w,z] = interp of polar_log_depths[b,h,w,:] at fixed log-spaced
                  positions  == polar[b,h,w,:] @ G      (G: [S,Z] const, 2 nnz/col)
  prob = softmax(scores, axis=h); cell = logsumexp(scores, axis=h)
  out[d,z,w] = sum_h image[b,d,h,w] * prob[h,w,z]

Sharding: data-parallel over batch B=32 across 8 cores (4 batches/core).
"""
import os
import sys

sys.path.insert(0, "/opt/trn_rl_repo")

from contextlib import ExitStack

import numpy as np

import concourse.bass as bass
import concourse.tile as tile
from concourse import bacc, mybir
from concourse.bass_utils import run_bass_kernel_spmd

N_CORES = 8
B, D, H, W, S = 32, 64, 64, 128, 64
Z = 64
BPC = B // N_CORES  # batches per core
FP = mybir.dt.float32
AF = mybir.ActivationFunctionType


def _g_matrix():
    """[S, Z] interp matrix: scores = P @ G."""
    z = np.arange(Z)
    pos = 10.5 * np.log2(1.0 + z)  # == norm * (S-1) for this config
    i0 = np.clip(np.floor(pos).astype(np.int64), 0, S - 1)
    i1 = np.clip(i0 + 1, 0, S - 1)
    wt = (pos - i0).astype(np.float32)
    G = np.zeros((S, Z), np.float32)
    np.add.at(G, (i0, z), 1.0 - wt)
    np.add.at(G, (i1, z), wt)
    return G


def _g2bd():
    """Block-diag over w-parity: [128=(w2,s), 128=(w2,z)]."""
    G = _g_matrix()
    out = np.zeros((128, 128), np.float32)
    out[:S, :Z] = G
    out[S:, Z:] = G
    return out


def _build_program(repeat=1):
    nc = bacc.Bacc("TRN2", target_bir_lowering=False, debug=False)
    img = nc.dram_tensor("img", [BPC, D, H, W], FP, kind="ExternalInput").ap()
    pol = nc.dram_tensor("pol", [BPC, H, W, S], FP, kind="ExternalInput").ap()
    g2bd = nc.dram_tensor("g2bd", [128, 128], FP, kind="ExternalInput").ap()
    ident = nc.dram_tensor("ident", [128, 128], FP, kind="ExternalInput").ap()
    ident2 = nc.dram_tensor("ident2", [128, 64], FP, kind="ExternalInput").ap()
    out = nc.dram_tensor("out", [BPC, D, Z, W], FP, kind="ExternalOutput").ap()
    cell = nc.dram_tensor("cell", [BPC, W, Z], FP, kind="ExternalOutput").ap()

    with tile.TileContext(nc) as tc, ExitStack() as ctx:
        consts = ctx.enter_context(tc.tile_pool(name="consts", bufs=1))
        share = ctx.enter_context(tc.tile_pool(name="share", bufs=1))
        imgt_pool = ctx.enter_context(tc.tile_pool(name="imgt", bufs=1))
        pt2_pool = ctx.enter_context(tc.tile_pool(name="pt2", bufs=1))
        e1_pool = ctx.enter_context(tc.tile_pool(name="e1", bufs=1))
        stage_pool = ctx.enter_context(tc.tile_pool(name="stage", bufs=2))
        small_pool = ctx.enter_context(tc.tile_pool(name="small", bufs=2))
        pt_ps = ctx.enter_context(tc.tile_pool(name="pt_ps", bufs=2, space="PSUM"))
        in_ps = ctx.enter_context(tc.tile_pool(name="in_ps", bufs=2, space="PSUM"))
        ei_ps = ctx.enter_context(tc.tile_pool(name="ei_ps", bufs=2, space="PSUM"))

        sb_g = consts.tile([128, 128], FP)
        nc.sync.dma_start(out=sb_g[:], in_=g2bd)
        sb_i = consts.tile([128, 128], FP)
        nc.sync.dma_start(out=sb_i[:], in_=ident)
        sb_i2 = consts.tile([128, 64], FP)  # [eye64; eye64] for 64-part strips
        nc.sync.dma_start(out=sb_i2[:], in_=ident2)

        for bp in range(repeat * (BPC // 2)):  # batch pairs
            b0 = 2 * (bp % (BPC // 2))

            # ---- loads ----
            # P2: [(b2 h) 128, (w s) 8192]  (fully dense both sides)
            P2 = share.tile([128, W * S], FP, tag="p2et")
            pol_src = pol[b0 : b0 + 2].rearrange("b h w s -> (b h) (w s)")
            for q in range(4):  # split for pipelining
                sl = slice(q * 2048, (q + 1) * 2048)
                nc.sync.dma_start(out=P2[:, sl], in_=pol_src[:, sl])
            # imgT2: [(b2 h) 128, (d w) 8192]  (transposed load, 512B chunks)
            IMGT = imgt_pool.tile([128, D * W], FP)
            for b2 in range(2):
                nc.sync.dma_start(
                    out=IMGT[b2 * 64 : b2 * 64 + 64, :],
                    in_=img[b0 + b2].transpose([1, 0, 2]),
                )

            # ---- stage B: transpose P -> PT' [(w2 s) 128, (wp b2 h) 8192] ----
            PT2 = pt2_pool.tile([128, 8192], FP)
            pt2_3d = PT2.rearrange("p (wp bh) -> p wp bh", wp=64)  # bh = (b2 h)
            for b2 in range(2):
                bsl = slice(b2 * 64, b2 * 64 + 64)
                for g in range(8):  # groups of 8 w-pairs per psum bank
                    ps = pt_ps.tile([128, 512], FP)
                    for j in range(8):
                        wp = g * 8 + j
                        nc.tensor.transpose(
                            ps[:, j * 64 : (j + 1) * 64],
                            P2[bsl, wp * 128 : (wp + 1) * 128],
                            sb_i2[bsl, :],
                        )
                    # strided evict: slot j -> col (g*8+j)*128 + b2*64
                    nc.scalar.copy(
                        pt2_3d[:, g * 8 : (g + 1) * 8, b2 * 64 : b2 * 64 + 64],
                        ps.rearrange("p (j h) -> p j h", j=8),
                    )

            # ---- stage C: interp matmul + fused exp evict ----
            # E1: [(w2 z) 128, (wp b2 h) 8192]
            E1 = e1_pool.tile([128, 8192], FP)
            for n in range(16):
                ps = in_ps.tile([128, 512], FP)
                nc.tensor.matmul(
                    ps[:], sb_g[:], PT2[:, n * 512 : (n + 1) * 512], start=True, stop=True
                )
                nc.scalar.activation(
                    E1[:, n * 512 : (n + 1) * 512], ps[:], func=AF.Exp
                )

            # ---- stage D: softmax denominators + cell score (per b2) ----
            e1_4d = E1.rearrange("p (wp b h) -> p wp b h", wp=64, b=2)
            for b2 in range(2):
                e1_b = e1_4d[:, :, b2, :]  # [128, 64(wp), 64(h)]
                Ssum = small_pool.tile([128, 64], FP)
                nc.vector.tensor_reduce(
                    Ssum[:], e1_b, axis=mybir.AxisListType.X, op=mybir.AluOpType.add
                )
                R = small_pool.tile([128, 64], FP)
                nc.vector.reciprocal(R[:], Ssum[:])
                r_b = R.unsqueeze(2).broadcast_to((128, 64, 64))
                nc.vector.tensor_mul(e1_b, e1_b, r_b)  # normalize in place
                Cl = small_pool.tile([128, 64], FP)
                nc.scalar.activation(Cl[:], Ssum[:], func=AF.Ln)
                psc = ei_ps.tile([128, 512], FP)
                nc.tensor.transpose(psc[0:64, 0:128], Cl[:], sb_i[:])
                CT = small_pool.tile([64, 128], FP)
                nc.vector.tensor_copy(CT[:], psc[0:64, 0:128])
                nc.sync.dma_start(
                    out=cell[b0 + b2].rearrange("(wp w2) z -> wp (w2 z)", w2=2),
                    in_=CT[:],
                )

            # ---- stage E: transpose prob -> ET [(b2 h) 128, (wp w2 z) 8192] ----
            # transpose input free dim spans (b2, h) via strided AP, so the
            # output partitions are (b2, h) directly.
            ET = share.tile([128, 8192], FP, tag="p2et")
            for g in range(16):  # groups of 4 wp per bank
                ps = pt_ps.tile([128, 512], FP)
                for j in range(4):
                    wp = g * 4 + j
                    nc.tensor.transpose(
                        ps[:, j * 128 : (j + 1) * 128],
                        E1[:, wp * 128 : (wp + 1) * 128],
                        sb_i[:],
                    )
                nc.scalar.copy(ET[:, g * 512 : (g + 1) * 512], ps[:])

            # ---- stage F: einsum out[d,z,w] = sum_h img * prob ----
            imgt_r = IMGT.rearrange("p (d w) -> p d w", w=W)
            for b2 in range(2):
                ST = stage_pool.tile([128, 4096], FP)  # [(whalf d), (z wj)]
                for bg in range(8):
                    ps = ei_ps.tile([128, 512], FP)
                    for whalf in range(2):
                        for j in range(8):
                            wj = bg * 8 + j
                            w = whalf * 64 + wj
                            wp, w2 = w >> 1, w & 1
                            lhsT = imgt_r[b2 * 64 : b2 * 64 + 64, :, w]
                            rhs = ET[
                                b2 * 64 : b2 * 64 + 64,
                                wp * 128 + w2 * 64 : wp * 128 + w2 * 64 + 64,
                            ]
                            nc.tensor.matmul(
                                ps[whalf * 64 : whalf * 64 + 64, j * 64 : (j + 1) * 64],
                                lhsT,
                                rhs,
                                start=True,
                                stop=True,
                            )
                    # evict, reordering (slot, z) -> (z, wj)
                    src = ps.rearrange("p (j z) -> p j z", j=8)
                    dst = ST.rearrange("p (z wj) -> p z wj", z=64)[
                        :, :, bg * 8 : (bg + 1) * 8
                    ].transpose([0, 2, 1])
                    if bg % 2 == 0:
                        nc.vector.tensor_copy(dst, src)
                    else:
                        nc.scalar.copy(dst, src)
                for wh in range(2):
                    nc.sync.dma_start(
                        out=out[b0 + b2]
                        .rearrange("d z (wh wj) -> d z wh wj", wh=2)
                        .transpose([2, 0, 1, 3])[wh],
                        in_=ST[wh * 64 : wh * 64 + 64, :],
                    )

    nc.compile()
    return nc


_NC = {}


def _get_nc(repeat=1):
    if repeat not in _NC:
        _NC[repeat] = _build_program(repeat)
    return _NC[repeat]


LAST_RESULTS = None


def kernel(image: np.ndarray, polar_log_depths: np.ndarray):
    global LAST_RESULTS
    nc = _get_nc()
    image = np.ascontiguousarray(image, dtype=np.float32)
    polar = np.ascontiguousarray(polar_log_depths, dtype=np.float32)
    g2 = _g2bd()
    ident = np.eye(128, dtype=np.float32)
    ident2 = np.concatenate([np.eye(64, dtype=np.float32)] * 2, axis=0)
    in_maps = []
    for c in range(N_CORES):
        sl = slice(c * BPC, (c + 1) * BPC)
        in_maps.append(
            {
                "img": image[sl],
                "pol": polar[sl],
                "g2bd": g2,
                "ident": ident,
                "ident2": ident2,
            }
        )
    trace = bool(int(os.environ.get("KERNEL_TRACE", "0")))
    res = run_bass_kernel_spmd(
        nc, in_maps, core_ids=list(range(N_CORES)), trace=trace
    )
    LAST_RESULTS = res
    image_polar = np.concatenate([res.results[c]["out"] for c in range(N_CORES)], 0)
    cell_score = np.concatenate([res.results[c]["cell"] for c in range(N_CORES)], 0)
    return image_polar, cell_score


# revision 33
# speedup vs baseline: 1.0575x; 1.0575x over previous
"""Trainium2 Bass kernel for PolarProjectionDepth.

reference computation (per batch b):
  scores[h,w,z] = interp of polar_log_depths[b,h,w,:] at fixed log-spaced
                  positions  == polar[b,h,w,:] @ G      (G: [S,Z] const, 2 nnz/col)
  prob = softmax(scores, axis=h); cell = logsumexp(scores, axis=h)
  out[d,z,w] = sum_h image[b,d,h,w] * prob[h,w,z]

Sharding: data-parallel over batch B=32 across 8 cores (4 batches/core).

Per-core dataflow (2 batch-pairs, all tiles 128 partitions):
  P2   [(b2 h), (w s)]      dense load
  PT'  [(w2 s), (wp b2 h)]  PE transposes (w-pair granularity)
  E1   [(w2 z), (wp b2 h)]  interp matmul (block-diag G) + fused exp evict
  softmax over h: per-512-slice reduce/recip/scale (h innermost free)
  ET   [(b2 h), (wp w2 z)]  PE transposes of normalized prob
  out  per (b,w) matmul [h,d]x[h,z], psum packed 2 w-halves x 8 slots/bank
"""
import os
import sys

sys.path.insert(0, "/opt/trn_rl_repo")

from contextlib import ExitStack

import numpy as np

import concourse.bass as bass
import concourse.tile as tile
from concourse import bacc, mybir
from concourse.bass_utils import run_bass_kernel_spmd

N_CORES = 8
B, D, H, W, S = 32, 64, 64, 128, 64
Z = 64
BPC = B // N_CORES  # batches per core
FP = mybir.dt.float32
AF = mybir.ActivationFunctionType


def _g_matrix():
    """[S, Z] interp matrix: scores = P @ G."""
    z = np.arange(Z)
    pos = 10.5 * np.log2(1.0 + z)  # == norm * (S-1) for this config
    i0 = np.clip(np.floor(pos).astype(np.int64), 0, S - 1)
    i1 = np.clip(i0 + 1, 0, S - 1)
    wt = (pos - i0).astype(np.float32)
    G = np.zeros((S, Z), np.float32)
    np.add.at(G, (i0, z), 1.0 - wt)
    np.add.at(G, (i1, z), wt)
    return G


def _g2bd():
    """Block-diag over w-parity: [128=(w2,s), 128=(w2,z)]."""
    G = _g_matrix()
    out = np.zeros((128, 128), np.float32)
    out[:S, :Z] = G
    out[S:, Z:] = G
    return out


def _build_program(repeat=1):
    nc = bacc.Bacc("TRN2", target_bir_lowering=False, debug=False)
    img = nc.dram_tensor("img", [BPC, D, H, W], FP, kind="ExternalInput").ap()
    pol = nc.dram_tensor("pol", [BPC, H, W, S], FP, kind="ExternalInput").ap()
    g2bd = nc.dram_tensor("g2bd", [128, 128], FP, kind="ExternalInput").ap()
    ident = nc.dram_tensor("ident", [128, 128], FP, kind="ExternalInput").ap()
    out = nc.dram_tensor("out", [BPC, D, Z, W], FP, kind="ExternalOutput").ap()
    cell = nc.dram_tensor("cell", [BPC, W, Z], FP, kind="ExternalOutput").ap()

    with tile.TileContext(nc) as tc, ExitStack() as ctx:
        consts = ctx.enter_context(tc.tile_pool(name="consts", bufs=1))
        share = ctx.enter_context(tc.tile_pool(name="share", bufs=1))
        imgt_pool = ctx.enter_context(tc.tile_pool(name="imgt", bufs=1))
        pt2_pool = ctx.enter_context(tc.tile_pool(name="pt2", bufs=1))
        e1_pool = ctx.enter_context(tc.tile_pool(name="e1", bufs=1))
        stage_pool = ctx.enter_context(tc.tile_pool(name="stage", bufs=2))
        small_pool = ctx.enter_context(tc.tile_pool(name="small", bufs=2))
        pt_ps = ctx.enter_context(tc.tile_pool(name="pt_ps", bufs=2, space="PSUM"))
        in_ps = ctx.enter_context(tc.tile_pool(name="in_ps", bufs=2, space="PSUM"))
        ei_ps = ctx.enter_context(tc.tile_pool(name="ei_ps", bufs=2, space="PSUM"))

        FPR = mybir.dt.float32r
        sb_g = consts.tile([128, 128], FP)
        nc.sync.dma_start(out=sb_g[:], in_=g2bd)
        sb_gr = consts.tile([128, 128], FPR)
        nc.scalar.copy(sb_gr[:], sb_g[:])  # round to fp32r once
        sb_i = consts.tile([128, 128], FP)
        nc.sync.dma_start(out=sb_i[:], in_=ident)

        for bp in range(repeat * (BPC // 2)):  # batch pairs
            b0 = 2 * (bp % (BPC // 2))

            # ---- loads ----
            P2 = share.tile([128, W * S], FP, tag="p2et")
            pol_src = pol[b0 : b0 + 2].rearrange("b h w s -> (b h) (w s)")
            for q in range(4):
                sl = slice(q * 2048, (q + 1) * 2048)
                nc.sync.dma_start(out=P2[:, sl], in_=pol_src[:, sl])
            IMGT = imgt_pool.tile([128, D * W], FP)
            for b2 in range(2):
                # SWDGE ring (Pool) — parallel to the SP HWDGE ring
                nc.gpsimd.dma_start(
                    out=IMGT[b2 * 64 : b2 * 64 + 64, :],
                    in_=img[b0 + b2].transpose([1, 0, 2]),
                )

            PT2 = pt2_pool.tile([128, 8192], FP)
            E1 = e1_pool.tile([128, 8192], FP)
            ET = share.tile([128, 8192], FP, tag="p2et")
            Ssum = small_pool.tile([128, 128], FP)  # cols (wp, b2)
            R = small_pool.tile([128, 128], FP)
            Cl0 = small_pool.tile([128, 64], FP)  # per-b2 log-sum, cols wp
            Cl1 = small_pool.tile([128, 64], FP)
            Cls = (Cl0, Cl1)
            e1_4d = E1.rearrange("p (wp b h) -> p wp b h", wp=64, b=2)

            for g in range(16):  # wp-groups of 4
                gsl = slice(g * 512, (g + 1) * 512)
                # stage B: transpose P w-pairs (both batches at once)
                ps_t = pt_ps.tile([128, 512], FP)
                for j in range(4):
                    wp = g * 4 + j
                    nc.tensor.transpose(
                        ps_t[:, j * 128 : (j + 1) * 128],
                        P2[:, wp * 128 : (wp + 1) * 128],
                        sb_i[:],
                    )
                nc.scalar.copy(PT2[:, gsl], ps_t[:])
                # stage C: interp matmul + fused exp evict
                ps_i = in_ps.tile([128, 512], FP)
                nc.tensor.matmul(
                    ps_i[:], sb_g[:], PT2[:, gsl], start=True, stop=True
                )
                nc.scalar.activation(E1[:, gsl], ps_i[:], func=AF.Exp)
                # stage D: per-slice softmax pieces
                ssl = slice(g * 8, (g + 1) * 8)
                nc.vector.tensor_reduce(
                    Ssum[:, ssl],
                    e1_4d[:, g * 4 : (g + 1) * 4, :, :],
                    axis=mybir.AxisListType.X,
                    op=mybir.AluOpType.add,
                )
                nc.vector.reciprocal(R[:, ssl], Ssum[:, ssl])
                r_b = (
                    R.rearrange("p (wp b) -> p wp b", wp=64)[:, g * 4 : (g + 1) * 4, :]
                    .unsqueeze(3)
                    .broadcast_to((128, 4, 2, 64))
                )
                e1_g = e1_4d[:, g * 4 : (g + 1) * 4, :, :]
                nc.vector.tensor_mul(e1_g, e1_g, r_b)  # normalize in place
                # stage E: transpose normalized prob
                ps_e = pt_ps.tile([128, 512], FP)
                for j in range(4):
                    wp = g * 4 + j
                    nc.tensor.transpose(
                        ps_e[:, j * 128 : (j + 1) * 128],
                        E1[:, wp * 128 : (wp + 1) * 128],
                        sb_i[:],
                    )
                nc.scalar.copy(ET[:, gsl], ps_e[:])

            # ---- cell score stores ----
            # Ln emitted once per pair (batching avoids ACT table thrash
            # between Exp and Ln sets)
            for b2 in range(2):
                nc.scalar.activation(
                    Cls[b2][:],
                    Ssum.rearrange("p (wp b) -> p wp b", wp=64)[:, :, b2],
                    func=AF.Ln,
                )
            for b2 in range(2):
                ps_c = ei_ps.tile([128, 512], FP, tag="eps")
                nc.tensor.transpose(ps_c[0:64, 0:128], Cls[b2][:], sb_i[:])
                CT = small_pool.tile([64, 128], FP)
                nc.vector.tensor_copy(CT[:], ps_c[0:64, 0:128])
                nc.sync.dma_start(
                    out=cell[b0 + b2].rearrange("(wp w2) z -> wp (w2 z)", w2=2),
                    in_=CT[:],
                )

            # ---- stage F: einsum ----
            imgt_r = IMGT.rearrange("p (d w) -> p d w", w=W)
            for b2 in range(2):
                ST = stage_pool.tile([128, 4096], FP)  # [(whalf d), (z wj)]
                for bg in range(8):
                    ps = ei_ps.tile([128, 512], FP, tag="eps")
                    for whalf in range(2):
                        for j in range(8):
                            wj = bg * 8 + j
                            w = whalf * 64 + wj
                            wp, w2 = w >> 1, w & 1
                            lhsT = imgt_r[b2 * 64 : b2 * 64 + 64, :, w]
                            rhs = ET[
                                b2 * 64 : b2 * 64 + 64,
                                wp * 128 + w2 * 64 : wp * 128 + w2 * 64 + 64,
                            ]
                            nc.tensor.matmul(
                                ps[whalf * 64 : whalf * 64 + 64, j * 64 : (j + 1) * 64],
                                lhsT,
                                rhs,
                                start=True,
                                stop=True,
                            )
                    src = ps.rearrange("p (j z) -> p j z", j=8)
                    dst = ST.rearrange("p (z wj) -> p z wj", z=64)[
                        :, :, bg * 8 : (bg + 1) * 8
                    ].transpose([0, 2, 1])
                    if bg % 2 == 0:
                        nc.vector.tensor_copy(dst, src)
                    else:
                        nc.scalar.copy(dst, src)
                for wh in range(2):
                    nc.sync.dma_start(
                        out=out[b0 + b2]
                        .rearrange("d z (wh wj) -> d z wh wj", wh=2)
                        .transpose([2, 0, 1, 3])[wh],
                        in_=ST[wh * 64 : wh * 64 + 64, :],
                    )

    nc.compile()
    return nc


_NC = {}


def _get_nc(repeat=1):
    if repeat not in _NC:
        _NC[repeat] = _build_program(repeat)
    return _NC[repeat]


LAST_RESULTS = None


def kernel(image: np.ndarray, polar_log_depths: np.ndarray):
    global LAST_RESULTS
    nc = _get_nc()
    image = np.ascontiguousarray(image, dtype=np.float32)
    polar = np.ascontiguousarray(polar_log_depths, dtype=np.float32)
    g2 = _g2bd()
    ident = np.eye(128, dtype=np.float32)
    in_maps = []
    for c in range(N_CORES):
        sl = slice(c * BPC, (c + 1) * BPC)
        in_maps.append(
            {"img": image[sl], "pol": polar[sl], "g2bd": g2, "ident": ident}
        )
    trace = bool(int(os.environ.get("KERNEL_TRACE", "0")))
    res = run_bass_kernel_spmd(
        nc, in_maps, core_ids=list(range(N_CORES)), trace=trace
    )
    LAST_RESULTS = res
    image_polar = np.concatenate([res.results[c]["out"] for c in range(N_CORES)], 0)
    cell_score = np.concatenate([res.results[c]["cell"] for c in range(N_CORES)], 0)
    return image_polar, cell_score


# revision 36
# speedup vs baseline: 1.0589x; 1.0013x over previous
"""Trainium2 Bass kernel for PolarProjectionDepth.

reference computation (per batch b):
  scores[h,w,z] = interp of polar_log_depths[b,h,w,:] at fixed log-spaced
                  positions  == polar[b,h,w,:] @ G      (G: [S,Z] const, 2 nnz/col)
  prob = softmax(scores, axis=h); cell = logsumexp(scores, axis=h)
  out[d,z,w] = sum_h image[b,d,h,w] * prob[h,w,z]

Sharding: data-parallel over batch B=32 across 8 cores (4 batches/core).

Per-core dataflow (2 batch-pairs, all tiles 128 partitions):
  P2   [(b2 h), (w s)]      dense load
  PT'  [(w2 s), (wp b2 h)]  PE transposes (w-pair granularity)
  E1   [(w2 z), (wp b2 h)]  interp matmul (block-diag G) + fused exp evict
  softmax over h: per-512-slice reduce/recip/scale (h innermost free)
  ET   [(b2 h), (wp w2 z)]  PE transposes of normalized prob
  out  per (b,w) matmul [h,d]x[h,z], psum packed 2 w-halves x 8 slots/bank
"""
import os
import sys

sys.path.insert(0, "/opt/trn_rl_repo")

from contextlib import ExitStack

import numpy as np

import concourse.bass as bass
import concourse.tile as tile
from concourse import bacc, mybir
from concourse.bass_utils import run_bass_kernel_spmd

N_CORES = 8
B, D, H, W, S = 32, 64, 64, 128, 64
Z = 64
BPC = B // N_CORES  # batches per core
FP = mybir.dt.float32
AF = mybir.ActivationFunctionType


def _g_matrix():
    """[S, Z] interp matrix: scores = P @ G."""
    z = np.arange(Z)
    pos = 10.5 * np.log2(1.0 + z)  # == norm * (S-1) for this config
    i0 = np.clip(np.floor(pos).astype(np.int64), 0, S - 1)
    i1 = np.clip(i0 + 1, 0, S - 1)
    wt = (pos - i0).astype(np.float32)
    G = np.zeros((S, Z), np.float32)
    np.add.at(G, (i0, z), 1.0 - wt)
    np.add.at(G, (i1, z), wt)
    return G


def _g2bd():
    """Block-diag over w-parity: [128=(w2,s), 128=(w2,z)]."""
    G = _g_matrix()
    out = np.zeros((128, 128), np.float32)
    out[:S, :Z] = G
    out[S:, Z:] = G
    return out


def _build_program(repeat=1):
    nc = bacc.Bacc("TRN2", target_bir_lowering=False, debug=False)
    img = nc.dram_tensor("img", [BPC, D, H, W], FP, kind="ExternalInput").ap()
    pol = nc.dram_tensor("pol", [BPC, H, W, S], FP, kind="ExternalInput").ap()
    g2bd = nc.dram_tensor("g2bd", [128, 128], FP, kind="ExternalInput").ap()
    ident = nc.dram_tensor("ident", [128, 128], FP, kind="ExternalInput").ap()
    out = nc.dram_tensor("out", [BPC, D, Z, W], FP, kind="ExternalOutput").ap()
    cell = nc.dram_tensor("cell", [BPC, W, Z], FP, kind="ExternalOutput").ap()

    with tile.TileContext(nc) as tc, ExitStack() as ctx:
        consts = ctx.enter_context(tc.tile_pool(name="consts", bufs=1))
        share = ctx.enter_context(tc.tile_pool(name="share", bufs=1))
        imgt_pool = ctx.enter_context(tc.tile_pool(name="imgt", bufs=1))
        pt2_pool = ctx.enter_context(tc.tile_pool(name="pt2", bufs=1))
        e1_pool = ctx.enter_context(tc.tile_pool(name="e1", bufs=1))
        stage_pool = ctx.enter_context(tc.tile_pool(name="stage", bufs=2))
        small_pool = ctx.enter_context(tc.tile_pool(name="small", bufs=2))
        pt_ps = ctx.enter_context(tc.tile_pool(name="pt_ps", bufs=4, space="PSUM"))
        in_ps = ctx.enter_context(tc.tile_pool(name="in_ps", bufs=2, space="PSUM"))
        ei_ps = ctx.enter_context(tc.tile_pool(name="ei_ps", bufs=2, space="PSUM"))

        FPR = mybir.dt.float32r
        sb_g = consts.tile([128, 128], FP)
        nc.sync.dma_start(out=sb_g[:], in_=g2bd)
        sb_gr = consts.tile([128, 128], FPR)
        nc.scalar.copy(sb_gr[:], sb_g[:])  # round to fp32r once
        sb_i = consts.tile([128, 128], FP)
        nc.sync.dma_start(out=sb_i[:], in_=ident)

        for bp in range(repeat * (BPC // 2)):  # batch pairs
            b0 = 2 * (bp % (BPC // 2))

            # ---- loads ----
            P2 = share.tile([128, W * S], FP, tag="p2et")
            pol_src = pol[b0 : b0 + 2].rearrange("b h w s -> (b h) (w s)")
            for q in range(4):
                sl = slice(q * 2048, (q + 1) * 2048)
                nc.sync.dma_start(out=P2[:, sl], in_=pol_src[:, sl])
            IMGT = imgt_pool.tile([128, D * W], FP)
            for b2 in range(2):
                # SWDGE ring (Pool) — parallel to the SP HWDGE ring
                nc.gpsimd.dma_start(
                    out=IMGT[b2 * 64 : b2 * 64 + 64, :],
                    in_=img[b0 + b2].transpose([1, 0, 2]),
                )

            PT2 = pt2_pool.tile([128, 8192], FP)
            E1 = e1_pool.tile([128, 8192], FP)
            ET = share.tile([128, 8192], FP, tag="p2et")
            Ssum = small_pool.tile([128, 128], FP)  # cols (wp, b2)
            R = small_pool.tile([128, 128], FP)
            Cl0 = small_pool.tile([128, 64], FP)  # per-b2 log-sum, cols wp
            Cl1 = small_pool.tile([128, 64], FP)
            Cls = (Cl0, Cl1)
            e1_4d = E1.rearrange("p (wp b h) -> p wp b h", wp=64, b=2)

            for g in range(16):  # wp-groups of 4
                gsl = slice(g * 512, (g + 1) * 512)
                # stage B: transpose P w-pairs (both batches at once)
                ps_t = pt_ps.tile([128, 512], FP, tag="tps")
                for j in range(4):
                    wp = g * 4 + j
                    nc.tensor.transpose(
                        ps_t[:, j * 128 : (j + 1) * 128],
                        P2[:, wp * 128 : (wp + 1) * 128],
                        sb_i[:],
                    )
                nc.scalar.copy(PT2[:, gsl], ps_t[:])
                # stage C: interp matmul + fused exp evict
                ps_i = in_ps.tile([128, 512], FP)
                nc.tensor.matmul(
                    ps_i[:], sb_g[:], PT2[:, gsl], start=True, stop=True
                )
                nc.scalar.activation(E1[:, gsl], ps_i[:], func=AF.Exp)
                # stage D: per-slice softmax pieces
                ssl = slice(g * 8, (g + 1) * 8)
                nc.vector.tensor_reduce(
                    Ssum[:, ssl],
                    e1_4d[:, g * 4 : (g + 1) * 4, :, :],
                    axis=mybir.AxisListType.X,
                    op=mybir.AluOpType.add,
                )
                nc.vector.reciprocal(R[:, ssl], Ssum[:, ssl])
                r_b = (
                    R.rearrange("p (wp b) -> p wp b", wp=64)[:, g * 4 : (g + 1) * 4, :]
                    .unsqueeze(3)
                    .broadcast_to((128, 4, 2, 64))
                )
                e1_g = e1_4d[:, g * 4 : (g + 1) * 4, :, :]
                nc.vector.tensor_mul(e1_g, e1_g, r_b)  # normalize in place
                # stage E: transpose normalized prob
                ps_e = pt_ps.tile([128, 512], FP, tag="tps")
                for j in range(4):
                    wp = g * 4 + j
                    nc.tensor.transpose(
                        ps_e[:, j * 128 : (j + 1) * 128],
                        E1[:, wp * 128 : (wp + 1) * 128],
                        sb_i[:],
                    )
                nc.scalar.copy(ET[:, gsl], ps_e[:])

            # ---- cell score stores ----
            # Ln emitted once per pair (batching avoids ACT table thrash
            # between Exp and Ln sets)
            for b2 in range(2):
                nc.scalar.activation(
                    Cls[b2][:],
                    Ssum.rearrange("p (wp b) -> p wp b", wp=64)[:, :, b2],
                    func=AF.Ln,
                )
            for b2 in range(2):
                ps_c = ei_ps.tile([128, 512], FP, tag="eps")
                nc.tensor.transpose(ps_c[0:64, 0:128], Cls[b2][:], sb_i[:])
                CT = small_pool.tile([64, 128], FP)
                nc.vector.tensor_copy(CT[:], ps_c[0:64, 0:128])
                nc.sync.dma_start(
                    out=cell[b0 + b2].rearrange("(wp w2) z -> wp (w2 z)", w2=2),
                    in_=CT[:],
                )

            # ---- stage F: einsum ----
            imgt_r = IMGT.rearrange("p (d w) -> p d w", w=W)
            for b2 in range(2):
                ST = stage_pool.tile([128, 4096], FP)  # [(whalf d), (z wj)]
                for bg in range(8):
                    ps = ei_ps.tile([128, 512], FP, tag="eps")
                    for whalf in range(2):
                        for j in range(8):
                            wj = bg * 8 + j
                            w = whalf * 64 + wj
                            wp, w2 = w >> 1, w & 1
                            lhsT = imgt_r[b2 * 64 : b2 * 64 + 64, :, w]
                            rhs = ET[
                                b2 * 64 : b2 * 64 + 64,
                                wp * 128 + w2 * 64 : wp * 128 + w2 * 64 + 64,
                            ]
                            nc.tensor.matmul(
                                ps[whalf * 64 : whalf * 64 + 64, j * 64 : (j + 1) * 64],
                                lhsT,
                                rhs,
                                start=True,
                                stop=True,
                            )
                    src = ps.rearrange("p (j z) -> p j z", j=8)
                    dst = ST.rearrange("p (z wj) -> p z wj", z=64)[
                        :, :, bg * 8 : (bg + 1) * 8
                    ].transpose([0, 2, 1])
                    if bg % 2 == 0:
                        nc.vector.tensor_copy(dst, src)
                    else:
                        nc.scalar.copy(dst, src)
                for wh in range(2):
                    nc.sync.dma_start(
                        out=out[b0 + b2]
                        .rearrange("d z (wh wj) -> d z wh wj", wh=2)
                        .transpose([2, 0, 1, 3])[wh],
                        in_=ST[wh * 64 : wh * 64 + 64, :],
                    )

    nc.compile()
    return nc


_NC = {}


def _get_nc(repeat=1):
    if repeat not in _NC:
        _NC[repeat] = _build_program(repeat)
    return _NC[repeat]


LAST_RESULTS = None


def kernel(image: np.ndarray, polar_log_depths: np.ndarray):
    global LAST_RESULTS
    nc = _get_nc()
    image = np.ascontiguousarray(image, dtype=np.float32)
    polar = np.ascontiguousarray(polar_log_depths, dtype=np.float32)
    g2 = _g2bd()
    ident = np.eye(128, dtype=np.float32)
    in_maps = []
    for c in range(N_CORES):
        sl = slice(c * BPC, (c + 1) * BPC)
        in_maps.append(
            {"img": image[sl], "pol": polar[sl], "g2bd": g2, "ident": ident}
        )
    trace = bool(int(os.environ.get("KERNEL_TRACE", "0")))
    res = run_bass_kernel_spmd(
        nc, in_maps, core_ids=list(range(N_CORES)), trace=trace
    )
    LAST_RESULTS = res
    image_polar = np.concatenate([res.results[c]["out"] for c in range(N_CORES)], 0)
    cell_score = np.concatenate([res.results[c]["cell"] for c in range(N_CORES)], 0)
    return image_polar, cell_score
